# revision 14
# baseline (speedup 1.0000x reference)
"""Causal single-head attention (N=4096, D=F=1024) on 8 TRN2 NeuronCores.

Causally load-balanced sequence sharding: core c owns the sixteen 32-row
query blocks {127-8g-c : g=0..15}. Keys (raw x rows here) are rotated by
32*c rows (junk zeros ahead, gated by a per-key `ones` tensor and zeroed
vp rows) so each core runs ONE uniform SPMD program in which key tile t is
matmul'd against a compile-time prefix of the query columns (width
512-32*(t//2)). Softmax normalization + output bias are applied host-side
on the unnormalized projected output (linear, so exact).

Weight folds (all host-side, f64):
  scores = q k^T = x (Wq^T Wk) x^T + [per-query consts: softmax-invariant,
  dropped] + c_j (per-key, c = x @ Wk^T bq) + const. So launch A computes
  only u = x @ M (M = Wq^T Wk) instead of separate q/k projections — half
  the projection FLOPs — and launch B matmuls u against raw x keys, adding
  SCALE*c_j via the exp activation's per-partition bias.
  vp = x @ W_eff with W_eff = Wv^T P^T folds the output projection through
  the value path; v/proj biases fold into a host-side constant pb_eff.

Launch B scores run as fp8 e4m3 DoubleRow matmuls: 256-deep contraction
per call at 1 col/cycle = 2x the bf16 FLOP rate (measured 216ns per
512-col call, same as bf16, double the work). AV + rowsum stay bf16.
Warmup matmuls bridge the engine preamble so the PE's HAM clock gate is
open when real work starts; DMA is spread over four trigger queues
(sync/scalar/gpsimd/vector).
"""

import sys

try:
    import concourse.bass as bass
except ImportError:  # pragma: no cover
    sys.path.insert(0, "/opt/trn_rl_repo")
    import concourse.bass as bass

import ml_dtypes
import numpy as np

import concourse.mybir as mybir
import concourse.tile as tile
from concourse import bacc
from concourse.bass_utils import run_bass_kernel_spmd

N, D, F = 4096, 1024, 1024
C = 8              # cores
NL = N // C        # 512 query rows per core
P = 128
SCALE = 1.0 / float(np.sqrt(np.float32(F)))

F32 = mybir.dt.float32
MM_DT = mybir.dt.bfloat16  # matmul operand dtype (PSUM accumulation stays f32)
QK_DT = mybir.dt.float8e4  # u/x score operands: e4m3, DoubleRow-capable
DR = mybir.MatmulPerfMode.DoubleRow

DT = D // P        # 8 contraction tiles
FT = F // P        # 8 f tiles
MT = N // P        # 32 key tiles
NT2 = NL // P      # 4 query-row tiles per core

WARMUP_A = 11
WARMUP_B = 11

# column width of key tile t (prefix of the query columns, 32-row blocks)
def _lw(t):
    return 512 - 32 * (t // 2)


# Filled with [launchA_ns, launchB_ns] when BASS_TRACE=1 profiling is active.
LAST_EXEC_NS = [None, None]
LAST_RESULTS = [None, None]

_CACHE = {}


def _build_uvp():
    nc = bacc.Bacc(None, target_bir_lowering=False)
    xT = nc.dram_tensor("xT", [P, DT, NL], MM_DT, kind="ExternalInput")
    # dt-major M blocks: mqb[dt][p, ft, m] = M[dt*128+p, ft*128+m]
    mqb = nc.dram_tensor("mqb", [DT, P, FT, P], MM_DT, kind="ExternalInput")
    wpb = nc.dram_tensor("wpb", [2, P, DT, 512], MM_DT, kind="ExternalInput")
    uT_o = nc.dram_tensor("uT_o", [F, NL], QK_DT, kind="ExternalOutput")
    vp_o = nc.dram_tensor("vp_o", [NL, F], MM_DT, kind="ExternalOutput")

    with tile.TileContext(nc) as tc:
        with (
            tc.tile_pool(name="singles", bufs=1) as singles,
            tc.tile_pool(name="weights", bufs=4) as weights,
            tc.tile_pool(name="osb", bufs=12) as opool,
            tc.tile_pool(name="upsum", bufs=1, space="PSUM") as upsum,
        ):
            warm = singles.tile([P, NL], MM_DT)
            nc.vector.memset(warm, 0.0)
            # u-phase psum accumulators double as warmup targets (dt-outer
            # accumulation across all 8 banks).
            upss = []
            for ft in range(FT):
                ups = upsum.tile([P, NL], F32, tag=f"u{ft}", name=f"ups{ft}")
                upss.append(ups)
            for wi in range(WARMUP_A):
                nc.tensor.matmul(
                    upss[0],
                    warm[:, :P],
                    warm,
                    start=(wi == 0),
                    stop=(wi == WARMUP_A - 1),
                )
            # ramp: first dt chunks of M and x land first so the dt-outer u
            # phase starts ~2us after DMA-go; wpb follows on the queues as
            # they free up.
            xT_sb = singles.tile([P, DT, NL], MM_DT)
            mdcs = []
            for dt_i in range(DT):
                mdc = weights.tile([P, FT, P], MM_DT, tag="mdc")
                mdcs.append(mdc)
            # interleave issue order: (mdc0, x0) (mdc1, x1) ... across queues
            engs = (nc.sync, nc.gpsimd, nc.scalar)
            for dt_i in range(DT):
                engs[dt_i % 3].dma_start(out=mdcs[dt_i], in_=mqb.ap()[dt_i])
                engs[(dt_i + 1) % 3].dma_start(
                    out=xT_sb[:, dt_i, :], in_=xT.ap()[:, dt_i, :]
                )
            wpcs = []
            for fc in range(2):
                wpc = weights.tile([P, DT, 512], MM_DT, tag=f"wpc{fc}")
                nc.sync.dma_start(out=wpc[:, : DT // 2, :], in_=wpb.ap()[fc, :, : DT // 2, :])
                nc.gpsimd.dma_start(out=wpc[:, DT // 2 :, :], in_=wpb.ap()[fc, :, DT // 2 :, :])
                wpcs.append(wpc)

            # u.T (dt-outer): 8 psum banks accumulate in parallel, so work
            # starts as soon as (mdc0, x0) land.
            for dt_i in range(DT):
                for ft in range(FT):
                    nc.tensor.matmul(
                        upss[ft],
                        mdcs[dt_i][:, ft, :],
                        xT_sb[:, dt_i, :],
                        start=(dt_i == 0),
                        stop=(dt_i == DT - 1),
                    )
            for ft in range(FT):
                osb = opool.tile([P, NL], QK_DT, tag="osb")
                nc.vector.tensor_copy(out=osb, in_=upss[ft])
                eng = (nc.gpsimd, nc.sync, nc.scalar)[ft % 3]
                eng.dma_start(out=uT_o.ap()[ft * P : (ft + 1) * P, :], in_=osb)

            # vp = v @ projT = x @ (wv^T projT): folded single matmul phase
            for fc in range(2):
                wpc = wpcs[fc]
                for mi in range(NT2):
                    ps = upsum.tile([P, 512], F32, tag=f"u{fc * NT2 + mi}")
                    for dt_i in range(DT):
                        nc.tensor.matmul(
                            ps,
                            xT_sb[:, dt_i, mi * P : (mi + 1) * P],
                            wpc[:, dt_i, :],
                            start=(dt_i == 0),
                            stop=(dt_i == DT - 1),
                        )
                    vsb = opool.tile([P, 512], MM_DT, tag="osb")
                    nc.vector.tensor_copy(out=vsb, in_=ps)
                    for half, eng in ((0, nc.scalar), (1, (nc.sync, nc.gpsimd)[mi % 2])):
                        hs = slice(fc * 512 + half * 256, fc * 512 + half * 256 + 256)
                        eng.dma_start(
                            out=vp_o.ap()[mi * P : (mi + 1) * P, hs],
                            in_=vsb[:, half * 256 : half * 256 + 256],
                        )
    nc.finalize()
    return nc


def _build_attn():
    nc = bacc.Bacc(None, target_bir_lowering=False)
    uT = nc.dram_tensor("uT", [P, FT, NL], QK_DT, kind="ExternalInput")
    xkb = nc.dram_tensor("xkb", [MT, P, FT, P], QK_DT, kind="ExternalInput")
    vbk = nc.dram_tensor("vbk", [FT, 2, P, MT // 2, P], MM_DT, kind="ExternalInput")
    ones = nc.dram_tensor("ones", [P, MT], MM_DT, kind="ExternalInput")
    cb = nc.dram_tensor("cb", [P, MT], F32, kind="ExternalInput")
    out_o = nc.dram_tensor("out_o", [F, NL], MM_DT, kind="ExternalOutput")
    rs_o = nc.dram_tensor("rs_o", [1, NL], F32, kind="ExternalOutput")

    with tile.TileContext(nc) as tc:
        with (
            tc.tile_pool(name="singles", bufs=1) as singles,
            tc.tile_pool(name="kc", bufs=12) as kpool,
            tc.tile_pool(name="vc", bufs=8) as vpool,
            tc.tile_pool(name="osb", bufs=3) as opool,
            tc.tile_pool(name="sps", bufs=3, space="PSUM") as spsum,
            tc.tile_pool(name="zps", bufs=3, space="PSUM") as zpsum,
            tc.tile_pool(name="rps", bufs=1, space="PSUM") as rpsum,
        ):
            warm = singles.tile([P, NL], MM_DT)
            nc.vector.memset(warm, 0.0)
            wps = spsum.tile([P, NL], F32, tag="sps")
            for wi in range(WARMUP_B):
                nc.tensor.matmul(
                    wps,
                    warm[:, :P],
                    warm,
                    start=(wi == 0),
                    stop=(wi == WARMUP_B - 1),
                )
            LOOKAHEAD = 10
            kcs = {}

            def _kc_dma(t, eng):
                kc = kpool.tile([P, FT, P], QK_DT, tag="kc")
                eng.dma_start(out=kc, in_=xkb.ap()[t])
                kcs[t] = kc

            # ramp: cb/ones + uT quarters on scalar (cb feeds the first exp;
            # quarter k feeds score ft pairs 2k/2k+1); xk tiles on sync/gpsimd.
            cb_sb = singles.tile([P, MT], F32)
            nc.scalar.dma_start(out=cb_sb, in_=cb.ap())
            ones_sb = singles.tile([P, MT], MM_DT)
            nc.scalar.dma_start(out=ones_sb, in_=ones.ap())
            uT_sb = singles.tile([P, FT, NL], QK_DT)
            for q in range(4):
                nc.scalar.dma_start(
                    out=uT_sb[:, 2 * q : 2 * q + 2, :],
                    in_=uT.ap()[:, 2 * q : 2 * q + 2, :],
                )
            _kc_dma(0, nc.sync)
            _kc_dma(1, nc.gpsimd)
            _kc_dma(2, nc.sync)
            _kc_dma(3, nc.gpsimd)
            _kc_dma(4, nc.sync)
            _kc_dma(5, nc.gpsimd)
            _kc_dma(6, nc.sync)
            _kc_dma(7, nc.gpsimd)
            _kc_dma(8, nc.sync)
            _kc_dma(9, nc.gpsimd)
            # first vp chunks prefetched (needed mid-kernel)
            vc_pre = []
            for vh in range(3):
                vc = vpool.tile([P, MT // 2, P], MM_DT, tag="vc")
                nc.scalar.dma_start(out=vc, in_=vbk.ap()[vh // 2, vh % 2])
                vc_pre.append(vc)

            # ---- scores + exp: pT[m, n] = exp(SCALE * (u x^T)[n, m] + SCALE*c_m)
            # fp8e4 DoubleRow: each call contracts an ft PAIR (256 rows).
            pts = []
            for t in range(MT):
                L = _lw(t)
                ta = t + LOOKAHEAD
                if ta < MT:
                    _kc_dma(ta, nc.sync if ta % 2 == 0 else nc.gpsimd)
                if t == 2:
                    vc = vpool.tile([P, MT // 2, P], MM_DT, tag="vc")
                    nc.scalar.dma_start(out=vc, in_=vbk.ap()[1, 1])
                    vc_pre.append(vc)
                kc = kcs.pop(t)
                ps = spsum.tile([P, NL], F32, tag="sps")
                for fi in range(0, FT, 2):
                    nc.tensor.matmul(
                        ps[:, :L],
                        kc[:, fi : fi + 2, :],
                        uT_sb[:, fi : fi + 2, :L],
                        start=(fi == 0),
                        stop=(fi == FT - 2),
                        perf_mode=DR,
                    )
                pt = singles.tile([P, NL], MM_DT, tag=f"pt{t}")
                nc.scalar.activation(
                    out=pt[:, :L],
                    in_=ps[:, :L],
                    func=mybir.ActivationFunctionType.Exp,
                    scale=SCALE,
                    bias=cb_sb[:, t : t + 1],
                )
                if t % 2 == 1:
                    # diagonal 128-key x 32-query block: keep key r <= 96 + n
                    nc.gpsimd.affine_select(
                        out=pt[:, L - 32 : L],
                        in_=pt[:, L - 32 : L],
                        pattern=[[1, 32]],
                        compare_op=mybir.AluOpType.is_ge,
                        fill=0.0,
                        base=96,
                        channel_multiplier=-1,
                    )
                pts.append(pt)

            # ---- row sums (junk tiles excluded via per-core ones data)
            rps = rpsum.tile([P, NL], F32, tag="rps")
            for t in range(MT):
                nc.tensor.matmul(
                    rps[0:1, : _lw(t)],
                    ones_sb[:, t : t + 1],
                    pts[t][:, : _lw(t)],
                    start=(t == 0),
                    stop=(t == MT - 1),
                )
            rs_sb = singles.tile([1, NL], F32)
            nc.vector.tensor_copy(out=rs_sb, in_=rps[0:1, :])
            nc.sync.dma_start(out=rs_o.ap(), in_=rs_sb)

            # ---- z.T[f, n] = sum_m vp[m, f] * pT[m, n]  (unnormalized)
            for ft in range(FT):
                for vh in range(2):  # half-chunks of 16 key tiles
                    if 2 * ft + vh < 4:
                        vc = vc_pre[2 * ft + vh]
                    else:
                        vc = vpool.tile([P, MT // 2, P], MM_DT, tag="vc")
                        eng = (nc.gpsimd, nc.scalar, nc.sync)[(2 * ft + vh) % 3]
                        eng.dma_start(out=vc, in_=vbk.ap()[ft, vh])
                    if vh == 0:
                        zps = zpsum.tile([P, NL], F32, tag="zps")
                    for mi in range(MT // 2):
                        t = vh * 16 + mi
                        L = _lw(t)
                        nc.tensor.matmul(
                            zps[:, :L],
                            vc[:, mi, :],
                            pts[t][:, :L],
                            start=(t == 0),
                            stop=(t == MT - 1),
                        )
                osb = opool.tile([P, NL], MM_DT, tag="osb")
                nc.vector.tensor_copy(out=osb, in_=zps)
                for half, eng in ((0, nc.scalar), (1, nc.sync)):
                    hs = slice(half * 256, half * 256 + 256)
                    eng.dma_start(
                        out=out_o.ap()[ft * P : (ft + 1) * P, hs],
                        in_=osb[:, hs],
                    )
    nc.finalize()
    return nc


def _get_programs():
    if "uvp" not in _CACHE:
        _CACHE["uvp"] = _build_uvp()
        _CACHE["attn"] = _build_attn()
    return _CACHE["uvp"], _CACHE["attn"]


def _c(a):
    return np.ascontiguousarray(a, dtype=np.float32)


def _b(a):
    return np.ascontiguousarray(np.asarray(a, dtype=np.float32).astype(ml_dtypes.bfloat16))


def _blocks_for_core(c):
    # 32-row query blocks, col group g owns block 127 - 8g - c
    return [127 - 8 * g - c for g in range(16)]


def kernel(x, wq_w, wq_b, wk_w, wk_b, wv_w, wv_b, proj_w, proj_b):
    x = np.asarray(x, dtype=np.float32)
    nc_uvp, nc_attn = _get_programs()

    # ---- host weight folds (f64)
    wq64 = np.asarray(wq_w, dtype=np.float64)
    wk64 = np.asarray(wk_w, dtype=np.float64)
    m_qk = wq64.T @ wk64                        # [D, D]: scores = x M x^T + ...
    c_key = x.astype(np.float64) @ (wk64.T @ np.asarray(wq_b, dtype=np.float64))
    w_eff = (
        np.asarray(wv_w, dtype=np.float64).T @ np.asarray(proj_w, dtype=np.float64).T
    )

    # ---- launch A: u = x @ M and vp = x @ W_eff, sequence-sharded
    xT = x.T                                    # [D, N]
    mqb = _b(m_qk.reshape(DT, P, FT, P))        # dt-major blocks
    wpb = _b(w_eff.reshape(DT, P, 2, 512).transpose(2, 1, 0, 3))
    in_a = []
    for c in range(C):
        xT_blk = _b(
            xT[:, c * NL : (c + 1) * NL].reshape(DT, P, NL).transpose(1, 0, 2)
        )
        in_a.append({"xT": xT_blk, "mqb": mqb, "wpb": wpb})
    res_a = run_bass_kernel_spmd(nc_uvp, in_a, core_ids=list(range(C)))
    LAST_EXEC_NS[0] = res_a.exec_time_ns
    LAST_RESULTS[0] = res_a

    uT_full = np.concatenate([res_a.results[c]["uT_o"] for c in range(C)], axis=1)
    vp_full = np.concatenate([res_a.results[c]["vp_o"] for c in range(C)], axis=0)

    # key-side fp8 x (full, shared): xT8[f, n] = x^T quantized e4m3
    xT8 = np.ascontiguousarray(xT.astype(ml_dtypes.float8_e4m3))

    # ---- launch B: attention, causally balanced
    in_b = []
    for c in range(C):
        qsel = np.concatenate(
            [uT_full[:, b * 32 : (b + 1) * 32] for b in _blocks_for_core(c)], axis=1
        )  # [F, NL]
        uT_blk = np.ascontiguousarray(qsel.reshape(FT, P, NL).transpose(1, 0, 2))
        # rotate keys/values by 32*c rows (junk zeros in rotated rows < 32c)
        sh = 32 * c
        xr = np.zeros((F, N), dtype=ml_dtypes.float8_e4m3)
        xr[:, sh:] = xT8[:, : N - sh]
        xkb_blk = np.ascontiguousarray(xr.reshape(FT, P, MT, P).transpose(2, 1, 0, 3))
        vr = np.zeros((N, F), dtype=ml_dtypes.bfloat16)
        vr[sh:, :] = vp_full[: N - sh]
        vbk_blk = np.ascontiguousarray(
            vr.reshape(2, MT // 2, P, FT, P).transpose(3, 0, 2, 1, 4)
        )
        cr = np.zeros(N, dtype=np.float64)
        cr[sh:] = c_key[: N - sh]
        cb_blk = _c(SCALE * cr.reshape(MT, P).T)   # [P, MT]
        # ones[r, t] gates key (128t + r); zero for junk rotated rows < 32c
        key_idx = np.arange(N).reshape(MT, P).T  # [P, MT]
        ones_blk = (key_idx >= sh).astype(ml_dtypes.bfloat16)
        in_b.append(
            {
                "uT": uT_blk,
                "xkb": xkb_blk,
                "vbk": vbk_blk,
                "ones": ones_blk,
                "cb": cb_blk,
            }
        )
    res_b = run_bass_kernel_spmd(nc_attn, in_b, core_ids=list(range(C)))
    LAST_EXEC_NS[1] = res_b.exec_time_ns
    LAST_RESULTS[1] = res_b

    # ---- host: unshuffle rows, normalize, add folded bias (linear => exact)
    # v-bias folds through attention exactly: att_norm @ (1 bv^T) projT = bv projT
    pb_eff = (
        np.asarray(proj_b, dtype=np.float64)
        + np.asarray(wv_b, dtype=np.float64) @ np.asarray(proj_w, dtype=np.float64).T
    ).astype(np.float32)
    out = np.empty((N, F), dtype=np.float32)
    for c in range(C):
        o_c = np.asarray(res_b.results[c]["out_o"], dtype=np.float32).T  # [NL, F]
        r_c = res_b.results[c]["rs_o"][0]        # [NL]
        for j, blk in enumerate(_blocks_for_core(c)):
            rows = o_c[j * 32 : (j + 1) * 32] / r_c[j * 32 : (j + 1) * 32, None]
            out[blk * 32 : (blk + 1) * 32] = rows + pb_eff
    return out


# revision 21
# speedup vs baseline: 1.0993x; 1.0993x over previous
"""Causal single-head attention (N=4096, D=F=1024) on 8 TRN2 NeuronCores.

Causally load-balanced sequence sharding: core c owns the sixteen 32-row
query blocks {127-8g-c : g=0..15}. Keys (raw x rows here) are rotated by
32*c rows (junk zeros ahead, gated by a per-key `ones` tensor and zeroed
vp rows) so each core runs ONE uniform SPMD program in which key tile t is
matmul'd against a compile-time prefix of the query columns (width
512-32*(t//2)). Softmax normalization + output bias are applied host-side
on the unnormalized projected output (linear, so exact).

Weight folds (all host-side, f64):
  scores = q k^T = x (Wq^T Wk) x^T + [per-query consts: softmax-invariant,
  dropped] + c_j (per-key, c = x @ Wk^T bq) + const. So launch A computes
  only u = x @ M (M = Wq^T Wk) instead of separate q/k projections — half
  the projection FLOPs — and launch B matmuls u against raw x keys, adding
  SCALE*c_j via the exp activation's per-partition bias.
  vp = x @ W_eff with W_eff = Wv^T P^T folds the output projection through
  the value path; v/proj biases fold into a host-side constant pb_eff.

Launch B scores run as fp8 e4m3 DoubleRow matmuls: 256-deep contraction
per call at 1 col/cycle = 2x the bf16 FLOP rate (measured 216ns per
512-col call, same as bf16, double the work). AV + rowsum stay bf16.
Warmup matmuls bridge the engine preamble so the PE's HAM clock gate is
open when real work starts; DMA is spread over four trigger queues
(sync/scalar/gpsimd/vector).
"""

import sys

try:
    import concourse.bass as bass
except ImportError:  # pragma: no cover
    sys.path.insert(0, "/opt/trn_rl_repo")
    import concourse.bass as bass

import ml_dtypes
import numpy as np

import concourse.mybir as mybir
import concourse.tile as tile
from concourse import bacc
from concourse.bass_utils import run_bass_kernel_spmd

N, D, F = 4096, 1024, 1024
C = 8              # cores
NL = N // C        # 512 query rows per core
P = 128
SCALE = 1.0 / float(np.sqrt(np.float32(F)))

F32 = mybir.dt.float32
MM_DT = mybir.dt.bfloat16  # matmul operand dtype (PSUM accumulation stays f32)
QK_DT = mybir.dt.float8e4  # u/x score operands: e4m3, DoubleRow-capable
DR = mybir.MatmulPerfMode.DoubleRow

DT = D // P        # 8 contraction tiles
FT = F // P        # 8 f tiles
MT = N // P        # 32 key tiles
NT2 = NL // P      # 4 query-row tiles per core

WARMUP_A = 11
WARMUP_B = 11

# column width of key tile t (prefix of the query columns, 32-row blocks)
def _lw(t):
    return 512 - 32 * (t // 2)


# Filled with [launchA_ns, launchB_ns] when BASS_TRACE=1 profiling is active.
LAST_EXEC_NS = [None, None]
LAST_RESULTS = [None, None]

_CACHE = {}


def _build_uvp():
    nc = bacc.Bacc(None, target_bir_lowering=False)
    xT = nc.dram_tensor("xT", [P, DT, NL], MM_DT, kind="ExternalInput")
    mqb = nc.dram_tensor("mqb", [FT, P, DT, P], MM_DT, kind="ExternalInput")
    wpb = nc.dram_tensor("wpb", [2, P, DT, 512], MM_DT, kind="ExternalInput")
    uT_o = nc.dram_tensor("uT_o", [F, NL], QK_DT, kind="ExternalOutput")
    vp_o = nc.dram_tensor("vp_o", [NL, F], MM_DT, kind="ExternalOutput")

    with tile.TileContext(nc) as tc:
        with (
            tc.tile_pool(name="singles", bufs=1) as singles,
            tc.tile_pool(name="weights", bufs=6) as weights,
            tc.tile_pool(name="osb", bufs=12) as opool,
            tc.tile_pool(name="psum", bufs=6, space="PSUM") as psum,
        ):
            warm = singles.tile([P, NL], MM_DT)
            nc.vector.memset(warm, 0.0)
            wps = psum.tile([P, NL], F32, tag="ps")
            for wi in range(WARMUP_A):
                nc.tensor.matmul(
                    wps,
                    warm[:, :P],
                    warm,
                    start=(wi == 0),
                    stop=(wi == WARMUP_A - 1),
                )
            # ramp around depth-1 DMA rings: x halves block sync/gpsimd
            # early (nothing else needs them), wpb[1] alone on scalar, mq
            # chunks follow, wpb[0] split at the back of sync/gpsimd.
            xT_sb = singles.tile([P, DT, NL], MM_DT)
            nc.sync.dma_start(out=xT_sb[:, : DT // 2, :], in_=xT.ap()[:, : DT // 2, :])
            nc.gpsimd.dma_start(
                out=xT_sb[:, DT // 2 :, :], in_=xT.ap()[:, DT // 2 :, :]
            )
            wpcs = []
            for fc in range(2):
                wpc = weights.tile([P, DT, 512], MM_DT, tag=f"wpc{fc}")
                wpcs.append(wpc)
            nc.scalar.dma_start(out=wpcs[1], in_=wpb.ap()[1])
            mcs = []
            for ft in range(FT):
                mc = weights.tile([P, DT, P], MM_DT, tag="mc")
                eng = nc.sync if ft % 2 == 0 else nc.gpsimd
                eng.dma_start(out=mc, in_=mqb.ap()[ft])
                mcs.append(mc)
            nc.sync.dma_start(out=wpcs[0][:, : DT // 2, :], in_=wpb.ap()[0, :, : DT // 2, :])
            nc.gpsimd.dma_start(out=wpcs[0][:, DT // 2 :, :], in_=wpb.ap()[0, :, DT // 2 :, :])

            # u.T : out[f_tile, n] = sum_d M[d, f] * xT[d, n]
            for ft in range(FT):
                mc = mcs[ft]
                ps = psum.tile([P, NL], F32, tag="ps")
                for dt_i in range(DT):
                    nc.tensor.matmul(
                        ps,
                        mc[:, dt_i, :],
                        xT_sb[:, dt_i, :],
                        start=(dt_i == 0),
                        stop=(dt_i == DT - 1),
                    )
                osb = opool.tile([P, NL], QK_DT, tag="osb")
                nc.vector.tensor_copy(out=osb, in_=ps)
                nc.scalar.dma_start(out=uT_o.ap()[ft * P : (ft + 1) * P, :], in_=osb)

            # vp = v @ projT = x @ (wv^T projT): folded single matmul phase
            for fc in range(2):
                wpc = wpcs[fc]
                for mi in range(NT2):
                    ps = psum.tile([P, 512], F32, tag="ps")
                    for dt_i in range(DT):
                        nc.tensor.matmul(
                            ps,
                            xT_sb[:, dt_i, mi * P : (mi + 1) * P],
                            wpc[:, dt_i, :],
                            start=(dt_i == 0),
                            stop=(dt_i == DT - 1),
                        )
                    vsb = opool.tile([P, 512], MM_DT, tag="osb")
                    nc.vector.tensor_copy(out=vsb, in_=ps)
                    for half, eng in ((0, nc.scalar), (1, (nc.sync, nc.gpsimd)[mi % 2])):
                        hs = slice(fc * 512 + half * 256, fc * 512 + half * 256 + 256)
                        eng.dma_start(
                            out=vp_o.ap()[mi * P : (mi + 1) * P, hs],
                            in_=vsb[:, half * 256 : half * 256 + 256],
                        )
    nc.finalize()
    return nc


def _build_attn():
    nc = bacc.Bacc(None, target_bir_lowering=False)
    uT = nc.dram_tensor("uT", [P, FT, NL], QK_DT, kind="ExternalInput")
    xkb = nc.dram_tensor("xkb", [MT, P, FT, P], QK_DT, kind="ExternalInput")
    vbk = nc.dram_tensor("vbk", [FT, 2, P, MT // 2, P], MM_DT, kind="ExternalInput")
    ones = nc.dram_tensor("ones", [P, MT], MM_DT, kind="ExternalInput")
    cb = nc.dram_tensor("cb", [P, MT], F32, kind="ExternalInput")
    out_o = nc.dram_tensor("out_o", [F, NL], MM_DT, kind="ExternalOutput")
    rs_o = nc.dram_tensor("rs_o", [1, NL], F32, kind="ExternalOutput")

    with tile.TileContext(nc) as tc:
        with (
            tc.tile_pool(name="singles", bufs=1) as singles,
            tc.tile_pool(name="kc", bufs=10) as kpool,
            tc.tile_pool(name="vc", bufs=6) as vpool,
            tc.tile_pool(name="osb", bufs=3) as opool,
            tc.tile_pool(name="sps", bufs=3, space="PSUM") as spsum,
            tc.tile_pool(name="zps", bufs=3, space="PSUM") as zpsum,
            tc.tile_pool(name="rps", bufs=1, space="PSUM") as rpsum,
        ):
            warm = singles.tile([P, NL], MM_DT)
            nc.vector.memset(warm, 0.0)
            wps = spsum.tile([P, NL], F32, tag="sps")
            for wi in range(WARMUP_B):
                nc.tensor.matmul(
                    wps,
                    warm[:, :P],
                    warm,
                    start=(wi == 0),
                    stop=(wi == WARMUP_B - 1),
                )
            LOOKAHEAD = 8
            kcs = {}

            def _kc_dma(t, eng):
                kc = kpool.tile([P, FT, P], QK_DT, tag="kc")
                eng.dma_start(out=kc, in_=xkb.ap()[t])
                kcs[t] = kc

            # ramp: cb/ones + uT quarters on scalar (cb feeds the first exp;
            # quarter k feeds score ft pairs 2k/2k+1); xk tiles on sync/gpsimd.
            cb_sb = singles.tile([P, MT], F32)
            nc.scalar.dma_start(out=cb_sb, in_=cb.ap())
            ones_sb = singles.tile([P, MT], MM_DT)
            nc.scalar.dma_start(out=ones_sb, in_=ones.ap())
            uT_sb = singles.tile([P, FT, NL], QK_DT)
            for q in range(4):
                nc.scalar.dma_start(
                    out=uT_sb[:, 2 * q : 2 * q + 2, :],
                    in_=uT.ap()[:, 2 * q : 2 * q + 2, :],
                )
            _kc_dma(0, nc.sync)
            _kc_dma(1, nc.gpsimd)
            _kc_dma(2, nc.sync)
            _kc_dma(3, nc.gpsimd)
            _kc_dma(4, nc.sync)
            _kc_dma(5, nc.gpsimd)
            _kc_dma(6, nc.sync)
            _kc_dma(7, nc.gpsimd)
            # first vp chunks prefetched (needed mid-kernel)
            vc_pre = []
            for vh in range(2):
                vc = vpool.tile([P, MT // 2, P], MM_DT, tag="vc")
                nc.scalar.dma_start(out=vc, in_=vbk.ap()[0, vh])
                vc_pre.append(vc)

            # ---- scores + exp: pT[m, n] = exp(SCALE * (u x^T)[n, m] + SCALE*c_m)
            # fp8e4 DoubleRow: each call contracts an ft PAIR (256 rows).
            pts = []
            for t in range(MT):
                L = _lw(t)
                ta = t + LOOKAHEAD
                if ta < MT:
                    _kc_dma(ta, nc.sync if ta % 2 == 0 else nc.gpsimd)
                kc = kcs.pop(t)
                ps = spsum.tile([P, NL], F32, tag="sps")
                for fi in range(0, FT, 2):
                    nc.tensor.matmul(
                        ps[:, :L],
                        kc[:, fi : fi + 2, :],
                        uT_sb[:, fi : fi + 2, :L],
                        start=(fi == 0),
                        stop=(fi == FT - 2),
                        perf_mode=DR,
                    )
                pt = singles.tile([P, NL], MM_DT, tag=f"pt{t}")
                nc.scalar.activation(
                    out=pt[:, :L],
                    in_=ps[:, :L],
                    func=mybir.ActivationFunctionType.Exp,
                    scale=SCALE,
                    bias=cb_sb[:, t : t + 1],
                )
                if t % 2 == 1:
                    # diagonal 128-key x 32-query block: keep key r <= 96 + n
                    nc.gpsimd.affine_select(
                        out=pt[:, L - 32 : L],
                        in_=pt[:, L - 32 : L],
                        pattern=[[1, 32]],
                        compare_op=mybir.AluOpType.is_ge,
                        fill=0.0,
                        base=96,
                        channel_multiplier=-1,
                    )
                pts.append(pt)

            # ---- row sums (junk tiles excluded via per-core ones data)
            rps = rpsum.tile([P, NL], F32, tag="rps")
            for t in range(MT):
                nc.tensor.matmul(
                    rps[0:1, : _lw(t)],
                    ones_sb[:, t : t + 1],
                    pts[t][:, : _lw(t)],
                    start=(t == 0),
                    stop=(t == MT - 1),
                )
            rs_sb = singles.tile([1, NL], F32)
            nc.vector.tensor_copy(out=rs_sb, in_=rps[0:1, :])
            nc.sync.dma_start(out=rs_o.ap(), in_=rs_sb)

            # ---- z.T[f, n] = sum_m vp[m, f] * pT[m, n]  (unnormalized)
            for ft in range(FT):
                for vh in range(2):  # half-chunks of 16 key tiles
                    if ft == 0:
                        vc = vc_pre[vh]
                    else:
                        vc = vpool.tile([P, MT // 2, P], MM_DT, tag="vc")
                        eng = (nc.gpsimd, nc.scalar, nc.sync)[(2 * ft + vh) % 3]
                        eng.dma_start(out=vc, in_=vbk.ap()[ft, vh])
                    if vh == 0:
                        zps = zpsum.tile([P, NL], F32, tag="zps")
                    for mi in range(MT // 2):
                        t = vh * 16 + mi
                        L = _lw(t)
                        nc.tensor.matmul(
                            zps[:, :L],
                            vc[:, mi, :],
                            pts[t][:, :L],
                            start=(t == 0),
                            stop=(t == MT - 1),
                        )
                osb = opool.tile([P, NL], MM_DT, tag="osb")
                nc.vector.tensor_copy(out=osb, in_=zps)
                for half, eng in ((0, nc.scalar), (1, nc.sync)):
                    hs = slice(half * 256, half * 256 + 256)
                    eng.dma_start(
                        out=out_o.ap()[ft * P : (ft + 1) * P, hs],
                        in_=osb[:, hs],
                    )
    nc.finalize()
    return nc


def _get_programs():
    if "uvp" not in _CACHE:
        _CACHE["uvp"] = _build_uvp()
        _CACHE["attn"] = _build_attn()
    return _CACHE["uvp"], _CACHE["attn"]


def _c(a):
    return np.ascontiguousarray(a, dtype=np.float32)


def _b(a):
    return np.ascontiguousarray(np.asarray(a, dtype=np.float32).astype(ml_dtypes.bfloat16))


def _blocks_for_core(c):
    # 32-row query blocks, col group g owns block 127 - 8g - c
    return [127 - 8 * g - c for g in range(16)]


def kernel(x, wq_w, wq_b, wk_w, wk_b, wv_w, wv_b, proj_w, proj_b):
    x = np.asarray(x, dtype=np.float32)
    nc_uvp, nc_attn = _get_programs()

    # ---- host weight folds (f64)
    wq64 = np.asarray(wq_w, dtype=np.float64)
    wk64 = np.asarray(wk_w, dtype=np.float64)
    m_qk = wq64.T @ wk64                        # [D, D]: scores = x M x^T + ...
    c_key = x.astype(np.float64) @ (wk64.T @ np.asarray(wq_b, dtype=np.float64))
    w_eff = (
        np.asarray(wv_w, dtype=np.float64).T @ np.asarray(proj_w, dtype=np.float64).T
    )

    # ---- launch A: u = x @ M and vp = x @ W_eff, sequence-sharded
    xT = x.T                                    # [D, N]
    mqb = _b(m_qk.reshape(DT, P, FT, P).transpose(2, 1, 0, 3))
    wpb = _b(w_eff.reshape(DT, P, 2, 512).transpose(2, 1, 0, 3))
    in_a = []
    for c in range(C):
        xT_blk = _b(
            xT[:, c * NL : (c + 1) * NL].reshape(DT, P, NL).transpose(1, 0, 2)
        )
        in_a.append({"xT": xT_blk, "mqb": mqb, "wpb": wpb})
    res_a = run_bass_kernel_spmd(nc_uvp, in_a, core_ids=list(range(C)))
    LAST_EXEC_NS[0] = res_a.exec_time_ns
    LAST_RESULTS[0] = res_a

    uT_full = np.concatenate([res_a.results[c]["uT_o"] for c in range(C)], axis=1)
    vp_full = np.concatenate([res_a.results[c]["vp_o"] for c in range(C)], axis=0)

    # key-side fp8 x (full, shared): xT8[f, n] = x^T quantized e4m3
    xT8 = np.ascontiguousarray(xT.astype(ml_dtypes.float8_e4m3))

    # ---- launch B: attention, causally balanced
    in_b = []
    for c in range(C):
        qsel = np.concatenate(
            [uT_full[:, b * 32 : (b + 1) * 32] for b in _blocks_for_core(c)], axis=1
        )  # [F, NL]
        uT_blk = np.ascontiguousarray(qsel.reshape(FT, P, NL).transpose(1, 0, 2))
        # rotate keys/values by 32*c rows (junk zeros in rotated rows < 32c)
        sh = 32 * c
        xr = np.zeros((F, N), dtype=ml_dtypes.float8_e4m3)
        xr[:, sh:] = xT8[:, : N - sh]
        xkb_blk = np.ascontiguousarray(xr.reshape(FT, P, MT, P).transpose(2, 1, 0, 3))
        vr = np.zeros((N, F), dtype=ml_dtypes.bfloat16)
        vr[sh:, :] = vp_full[: N - sh]
        vbk_blk = np.ascontiguousarray(
            vr.reshape(2, MT // 2, P, FT, P).transpose(3, 0, 2, 1, 4)
        )
        cr = np.zeros(N, dtype=np.float64)
        cr[sh:] = c_key[: N - sh]
        cb_blk = _c(SCALE * cr.reshape(MT, P).T)   # [P, MT]
        # ones[r, t] gates key (128t + r); zero for junk rotated rows < 32c
        key_idx = np.arange(N).reshape(MT, P).T  # [P, MT]
        ones_blk = (key_idx >= sh).astype(ml_dtypes.bfloat16)
        in_b.append(
            {
                "uT": uT_blk,
                "xkb": xkb_blk,
                "vbk": vbk_blk,
                "ones": ones_blk,
                "cb": cb_blk,
            }
        )
    res_b = run_bass_kernel_spmd(nc_attn, in_b, core_ids=list(range(C)))
    LAST_EXEC_NS[1] = res_b.exec_time_ns
    LAST_RESULTS[1] = res_b

    # ---- host: unshuffle rows, normalize, add folded bias (linear => exact)
    # v-bias folds through attention exactly: att_norm @ (1 bv^T) projT = bv projT
    pb_eff = (
        np.asarray(proj_b, dtype=np.float64)
        + np.asarray(wv_b, dtype=np.float64) @ np.asarray(proj_w, dtype=np.float64).T
    ).astype(np.float32)
    out = np.empty((N, F), dtype=np.float32)
    for c in range(C):
        o_c = np.asarray(res_b.results[c]["out_o"], dtype=np.float32).T  # [NL, F]
        r_c = res_b.results[c]["rs_o"][0]        # [NL]
        for j, blk in enumerate(_blocks_for_core(c)):
            rows = o_c[j * 32 : (j + 1) * 32] / r_c[j * 32 : (j + 1) * 32, None]
            out[blk * 32 : (blk + 1) * 32] = rows + pb_eff
    return out


# revision 22
# speedup vs baseline: 1.1895x; 1.0820x over previous
"""Causal single-head attention (N=4096, D=F=1024) on 8 TRN2 NeuronCores.

Causally load-balanced sequence sharding: core c owns the sixteen 32-row
query blocks {127-8g-c : g=0..15}. Keys (raw x rows here) are rotated by
32*c rows (junk zeros ahead, gated by a per-key `ones` tensor and zeroed
vp rows) so each core runs ONE uniform SPMD program in which key tile t is
matmul'd against a compile-time prefix of the query columns (width
512-32*(t//2)). Softmax normalization + output bias are applied host-side
on the unnormalized projected output (linear, so exact).

Weight folds (all host-side, f64):
  scores = q k^T = x (Wq^T Wk) x^T + [per-query consts: softmax-invariant,
  dropped] + c_j (per-key, c = x @ Wk^T bq) + const. So launch A computes
  only u = x @ M (M = Wq^T Wk) instead of separate q/k projections — half
  the projection FLOPs — and launch B matmuls u against raw x keys, adding
  SCALE*c_j via the exp activation's per-partition bias.
  vp = x @ W_eff with W_eff = Wv^T P^T folds the output projection through
  the value path; v/proj biases fold into a host-side constant pb_eff.

Launch B scores run as fp8 e4m3 DoubleRow matmuls: 256-deep contraction
per call at 1 col/cycle = 2x the bf16 FLOP rate (measured 216ns per
512-col call, same as bf16, double the work). AV + rowsum stay bf16.
Warmup matmuls bridge the engine preamble so the PE's HAM clock gate is
open when real work starts; DMA is spread over four trigger queues
(sync/scalar/gpsimd/vector).
"""

import sys

try:
    import concourse.bass as bass
except ImportError:  # pragma: no cover
    sys.path.insert(0, "/opt/trn_rl_repo")
    import concourse.bass as bass

import ml_dtypes
import numpy as np

import concourse.mybir as mybir
import concourse.tile as tile
from concourse import bacc
from concourse.bass_utils import run_bass_kernel_spmd

N, D, F = 4096, 1024, 1024
C = 8              # cores
NL = N // C        # 512 query rows per core
P = 128
SCALE = 1.0 / float(np.sqrt(np.float32(F)))

F32 = mybir.dt.float32
MM_DT = mybir.dt.bfloat16  # matmul operand dtype (PSUM accumulation stays f32)
QK_DT = mybir.dt.float8e4  # u/x score operands: e4m3, DoubleRow-capable
DR = mybir.MatmulPerfMode.DoubleRow

DT = D // P        # 8 contraction tiles
FT = F // P        # 8 f tiles
MT = N // P        # 32 key tiles
NT2 = NL // P      # 4 query-row tiles per core

WARMUP_A = 11
WARMUP_B = 11

# column width of key tile t (prefix of the query columns, 32-row blocks)
def _lw(t):
    return 512 - 32 * (t // 2)


# Filled with [launchA_ns, launchB_ns] when BASS_TRACE=1 profiling is active.
LAST_EXEC_NS = [None, None]
LAST_RESULTS = [None, None]

_CACHE = {}


def _build_uvp():
    nc = bacc.Bacc(None, target_bir_lowering=False)
    xT = nc.dram_tensor("xT", [P, DT, NL], MM_DT, kind="ExternalInput")
    mqb = nc.dram_tensor("mqb", [FT, P, DT, P], MM_DT, kind="ExternalInput")
    wpb = nc.dram_tensor("wpb", [2, P, DT, 512], MM_DT, kind="ExternalInput")
    uT_o = nc.dram_tensor("uT_o", [F, NL], QK_DT, kind="ExternalOutput")
    vp_o = nc.dram_tensor("vp_o", [NL, F], MM_DT, kind="ExternalOutput")

    with tile.TileContext(nc) as tc:
        with (
            tc.tile_pool(name="singles", bufs=1) as singles,
            tc.tile_pool(name="weights", bufs=6) as weights,
            tc.tile_pool(name="osb", bufs=12) as opool,
            tc.tile_pool(name="psum", bufs=6, space="PSUM") as psum,
        ):
            warm = singles.tile([P, NL], MM_DT)
            nc.vector.memset(warm, 0.0)
            wps = psum.tile([P, NL], F32, tag="ps")
            for wi in range(WARMUP_A):
                nc.tensor.matmul(
                    wps,
                    warm[:, :P],
                    warm,
                    start=(wi == 0),
                    stop=(wi == WARMUP_A - 1),
                )
            # ramp around depth-1 DMA rings: x halves block sync/gpsimd
            # early (nothing else needs them), wpb[1] alone on scalar, mq
            # chunks follow, wpb[0] split at the back of sync/gpsimd.
            xT_sb = singles.tile([P, DT, NL], MM_DT)
            nc.sync.dma_start(out=xT_sb[:, : DT // 2, :], in_=xT.ap()[:, : DT // 2, :])
            nc.gpsimd.dma_start(
                out=xT_sb[:, DT // 2 :, :], in_=xT.ap()[:, DT // 2 :, :]
            )
            wpcs = []
            for fc in range(2):
                wpc = weights.tile([P, DT, 512], MM_DT, tag=f"wpc{fc}")
                wpcs.append(wpc)
            nc.scalar.dma_start(out=wpcs[1], in_=wpb.ap()[1])
            mcs = []
            for ft in range(FT):
                mc = weights.tile([P, DT, P], MM_DT, tag="mc")
                eng = nc.sync if ft % 2 == 0 else nc.gpsimd
                eng.dma_start(out=mc, in_=mqb.ap()[ft])
                mcs.append(mc)
            nc.sync.dma_start(out=wpcs[0][:, : DT // 2, :], in_=wpb.ap()[0, :, : DT // 2, :])
            nc.gpsimd.dma_start(out=wpcs[0][:, DT // 2 :, :], in_=wpb.ap()[0, :, DT // 2 :, :])

            # u.T : out[f_tile, n] = sum_d M[d, f] * xT[d, n]
            for ft in range(FT):
                mc = mcs[ft]
                ps = psum.tile([P, NL], F32, tag="ps")
                for dt_i in range(DT):
                    nc.tensor.matmul(
                        ps,
                        mc[:, dt_i, :],
                        xT_sb[:, dt_i, :],
                        start=(dt_i == 0),
                        stop=(dt_i == DT - 1),
                    )
                osb = opool.tile([P, NL], QK_DT, tag="osb")
                nc.vector.tensor_copy(out=osb, in_=ps)
                nc.scalar.dma_start(out=uT_o.ap()[ft * P : (ft + 1) * P, :], in_=osb)

            # vp = v @ projT = x @ (wv^T projT): folded single matmul phase
            for fc in range(2):
                wpc = wpcs[fc]
                for mi in range(NT2):
                    ps = psum.tile([P, 512], F32, tag="ps")
                    for dt_i in range(DT):
                        nc.tensor.matmul(
                            ps,
                            xT_sb[:, dt_i, mi * P : (mi + 1) * P],
                            wpc[:, dt_i, :],
                            start=(dt_i == 0),
                            stop=(dt_i == DT - 1),
                        )
                    vsb = opool.tile([P, 512], MM_DT, tag="osb")
                    nc.vector.tensor_copy(out=vsb, in_=ps)
                    for half, eng in ((0, nc.scalar), (1, (nc.sync, nc.gpsimd)[mi % 2])):
                        hs = slice(fc * 512 + half * 256, fc * 512 + half * 256 + 256)
                        eng.dma_start(
                            out=vp_o.ap()[mi * P : (mi + 1) * P, hs],
                            in_=vsb[:, half * 256 : half * 256 + 256],
                        )
    nc.finalize()
    return nc


def _build_attn():
    nc = bacc.Bacc(None, target_bir_lowering=False)
    uT = nc.dram_tensor("uT", [P, FT, NL], QK_DT, kind="ExternalInput")
    xkb = nc.dram_tensor("xkb", [MT, P, FT, P], QK_DT, kind="ExternalInput")
    vbk = nc.dram_tensor("vbk", [FT, 2, P, MT // 2, P], MM_DT, kind="ExternalInput")
    ones = nc.dram_tensor("ones", [P, MT], MM_DT, kind="ExternalInput")
    cb = nc.dram_tensor("cb", [P, MT], F32, kind="ExternalInput")
    out_o = nc.dram_tensor("out_o", [F, NL], MM_DT, kind="ExternalOutput")
    rs_o = nc.dram_tensor("rs_o", [1, NL], F32, kind="ExternalOutput")

    with tile.TileContext(nc) as tc:
        with (
            tc.tile_pool(name="singles", bufs=1) as singles,
            tc.tile_pool(name="kc", bufs=10) as kpool,
            tc.tile_pool(name="vc", bufs=6) as vpool,
            tc.tile_pool(name="osb", bufs=3) as opool,
            tc.tile_pool(name="sps", bufs=3, space="PSUM") as spsum,
            tc.tile_pool(name="zps", bufs=3, space="PSUM") as zpsum,
            tc.tile_pool(name="rps", bufs=1, space="PSUM") as rpsum,
        ):
            warm = singles.tile([P, NL], MM_DT)
            nc.vector.memset(warm, 0.0)
            wps = spsum.tile([P, NL], F32, tag="sps")
            for wi in range(WARMUP_B):
                nc.tensor.matmul(
                    wps,
                    warm[:, :P],
                    warm,
                    start=(wi == 0),
                    stop=(wi == WARMUP_B - 1),
                )
            LOOKAHEAD = 8
            kcs = {}

            def _kc_dma(t, eng):
                kc = kpool.tile([P, FT, P], QK_DT, tag="kc")
                eng.dma_start(out=kc, in_=xkb.ap()[t])
                kcs[t] = kc

            # ramp: cb/ones + uT quarters on scalar (cb feeds the first exp;
            # quarter k feeds score ft pairs 2k/2k+1); xk tiles on sync/gpsimd.
            cb_sb = singles.tile([P, MT], F32)
            nc.scalar.dma_start(out=cb_sb, in_=cb.ap())
            ones_sb = singles.tile([P, MT], MM_DT)
            nc.scalar.dma_start(out=ones_sb, in_=ones.ap())
            uT_sb = singles.tile([P, FT, NL], QK_DT)
            nc.scalar.dma_start(out=uT_sb, in_=uT.ap())
            _kc_dma(0, nc.sync)
            _kc_dma(1, nc.gpsimd)
            _kc_dma(2, nc.sync)
            _kc_dma(3, nc.gpsimd)
            _kc_dma(4, nc.sync)
            _kc_dma(5, nc.gpsimd)
            _kc_dma(6, nc.sync)
            _kc_dma(7, nc.gpsimd)
            # first vp chunks prefetched (needed mid-kernel)
            vc_pre = []
            for vh in range(2):
                vc = vpool.tile([P, MT // 2, P], MM_DT, tag="vc")
                nc.scalar.dma_start(out=vc, in_=vbk.ap()[0, vh])
                vc_pre.append(vc)

            # ---- scores + exp: pT[m, n] = exp(SCALE * (u x^T)[n, m] + SCALE*c_m)
            # fp8e4 DoubleRow: each call contracts an ft PAIR (256 rows).
            pts = []
            for t in range(MT):
                L = _lw(t)
                ta = t + LOOKAHEAD
                if ta < MT:
                    _kc_dma(ta, nc.sync if ta % 2 == 0 else nc.gpsimd)
                kc = kcs.pop(t)
                ps = spsum.tile([P, NL], F32, tag="sps")
                for fi in range(0, FT, 2):
                    nc.tensor.matmul(
                        ps[:, :L],
                        kc[:, fi : fi + 2, :],
                        uT_sb[:, fi : fi + 2, :L],
                        start=(fi == 0),
                        stop=(fi == FT - 2),
                        perf_mode=DR,
                    )
                pt = singles.tile([P, NL], MM_DT, tag=f"pt{t}")
                nc.scalar.activation(
                    out=pt[:, :L],
                    in_=ps[:, :L],
                    func=mybir.ActivationFunctionType.Exp,
                    scale=SCALE,
                    bias=cb_sb[:, t : t + 1],
                )
                if t % 2 == 1:
                    # diagonal 128-key x 32-query block: keep key r <= 96 + n
                    nc.gpsimd.affine_select(
                        out=pt[:, L - 32 : L],
                        in_=pt[:, L - 32 : L],
                        pattern=[[1, 32]],
                        compare_op=mybir.AluOpType.is_ge,
                        fill=0.0,
                        base=96,
                        channel_multiplier=-1,
                    )
                pts.append(pt)

            # ---- row sums (junk tiles excluded via per-core ones data)
            rps = rpsum.tile([P, NL], F32, tag="rps")
            for t in range(MT):
                nc.tensor.matmul(
                    rps[0:1, : _lw(t)],
                    ones_sb[:, t : t + 1],
                    pts[t][:, : _lw(t)],
                    start=(t == 0),
                    stop=(t == MT - 1),
                )
            rs_sb = singles.tile([1, NL], F32)
            nc.vector.tensor_copy(out=rs_sb, in_=rps[0:1, :])
            nc.sync.dma_start(out=rs_o.ap(), in_=rs_sb)

            # ---- z.T[f, n] = sum_m vp[m, f] * pT[m, n]  (unnormalized)
            for ft in range(FT):
                for vh in range(2):  # half-chunks of 16 key tiles
                    if ft == 0:
                        vc = vc_pre[vh]
                    else:
                        vc = vpool.tile([P, MT // 2, P], MM_DT, tag="vc")
                        eng = (nc.gpsimd, nc.scalar, nc.sync)[(2 * ft + vh) % 3]
                        eng.dma_start(out=vc, in_=vbk.ap()[ft, vh])
                    if vh == 0:
                        zps = zpsum.tile([P, NL], F32, tag="zps")
                    for mi in range(MT // 2):
                        t = vh * 16 + mi
                        L = _lw(t)
                        nc.tensor.matmul(
                            zps[:, :L],
                            vc[:, mi, :],
                            pts[t][:, :L],
                            start=(t == 0),
                            stop=(t == MT - 1),
                        )
                osb = opool.tile([P, NL], MM_DT, tag="osb")
                nc.vector.tensor_copy(out=osb, in_=zps)
                for half, eng in ((0, nc.scalar), (1, nc.sync)):
                    hs = slice(half * 256, half * 256 + 256)
                    eng.dma_start(
                        out=out_o.ap()[ft * P : (ft + 1) * P, hs],
                        in_=osb[:, hs],
                    )
    nc.finalize()
    return nc


def _get_programs():
    if "uvp" not in _CACHE:
        _CACHE["uvp"] = _build_uvp()
        _CACHE["attn"] = _build_attn()
    return _CACHE["uvp"], _CACHE["attn"]


def _c(a):
    return np.ascontiguousarray(a, dtype=np.float32)


def _b(a):
    return np.ascontiguousarray(np.asarray(a, dtype=np.float32).astype(ml_dtypes.bfloat16))


def _blocks_for_core(c):
    # 32-row query blocks, col group g owns block 127 - 8g - c
    return [127 - 8 * g - c for g in range(16)]


def kernel(x, wq_w, wq_b, wk_w, wk_b, wv_w, wv_b, proj_w, proj_b):
    x = np.asarray(x, dtype=np.float32)
    nc_uvp, nc_attn = _get_programs()

    # ---- host weight folds (f64)
    wq64 = np.asarray(wq_w, dtype=np.float64)
    wk64 = np.asarray(wk_w, dtype=np.float64)
    m_qk = wq64.T @ wk64                        # [D, D]: scores = x M x^T + ...
    c_key = x.astype(np.float64) @ (wk64.T @ np.asarray(wq_b, dtype=np.float64))
    w_eff = (
        np.asarray(wv_w, dtype=np.float64).T @ np.asarray(proj_w, dtype=np.float64).T
    )

    # ---- launch A: u = x @ M and vp = x @ W_eff, sequence-sharded
    xT = x.T                                    # [D, N]
    mqb = _b(m_qk.reshape(DT, P, FT, P).transpose(2, 1, 0, 3))
    wpb = _b(w_eff.reshape(DT, P, 2, 512).transpose(2, 1, 0, 3))
    in_a = []
    for c in range(C):
        xT_blk = _b(
            xT[:, c * NL : (c + 1) * NL].reshape(DT, P, NL).transpose(1, 0, 2)
        )
        in_a.append({"xT": xT_blk, "mqb": mqb, "wpb": wpb})
    res_a = run_bass_kernel_spmd(nc_uvp, in_a, core_ids=list(range(C)))
    LAST_EXEC_NS[0] = res_a.exec_time_ns
    LAST_RESULTS[0] = res_a

    uT_full = np.concatenate([res_a.results[c]["uT_o"] for c in range(C)], axis=1)
    vp_full = np.concatenate([res_a.results[c]["vp_o"] for c in range(C)], axis=0)

    # key-side fp8 x (full, shared): xT8[f, n] = x^T quantized e4m3
    xT8 = np.ascontiguousarray(xT.astype(ml_dtypes.float8_e4m3))

    # ---- launch B: attention, causally balanced
    in_b = []
    for c in range(C):
        qsel = np.concatenate(
            [uT_full[:, b * 32 : (b + 1) * 32] for b in _blocks_for_core(c)], axis=1
        )  # [F, NL]
        uT_blk = np.ascontiguousarray(qsel.reshape(FT, P, NL).transpose(1, 0, 2))
        # rotate keys/values by 32*c rows (junk zeros in rotated rows < 32c)
        sh = 32 * c
        xr = np.zeros((F, N), dtype=ml_dtypes.float8_e4m3)
        xr[:, sh:] = xT8[:, : N - sh]
        xkb_blk = np.ascontiguousarray(xr.reshape(FT, P, MT, P).transpose(2, 1, 0, 3))
        vr = np.zeros((N, F), dtype=ml_dtypes.bfloat16)
        vr[sh:, :] = vp_full[: N - sh]
        vbk_blk = np.ascontiguousarray(
            vr.reshape(2, MT // 2, P, FT, P).transpose(3, 0, 2, 1, 4)
        )
        cr = np.zeros(N, dtype=np.float64)
        cr[sh:] = c_key[: N - sh]
        cb_blk = _c(SCALE * cr.reshape(MT, P).T)   # [P, MT]
        # ones[r, t] gates key (128t + r); zero for junk rotated rows < 32c
        key_idx = np.arange(N).reshape(MT, P).T  # [P, MT]
        ones_blk = (key_idx >= sh).astype(ml_dtypes.bfloat16)
        in_b.append(
            {
                "uT": uT_blk,
                "xkb": xkb_blk,
                "vbk": vbk_blk,
                "ones": ones_blk,
                "cb": cb_blk,
            }
        )
    res_b = run_bass_kernel_spmd(nc_attn, in_b, core_ids=list(range(C)))
    LAST_EXEC_NS[1] = res_b.exec_time_ns
    LAST_RESULTS[1] = res_b

    # ---- host: unshuffle rows, normalize, add folded bias (linear => exact)
    # v-bias folds through attention exactly: att_norm @ (1 bv^T) projT = bv projT
    pb_eff = (
        np.asarray(proj_b, dtype=np.float64)
        + np.asarray(wv_b, dtype=np.float64) @ np.asarray(proj_w, dtype=np.float64).T
    ).astype(np.float32)
    out = np.empty((N, F), dtype=np.float32)
    for c in range(C):
        o_c = np.asarray(res_b.results[c]["out_o"], dtype=np.float32).T  # [NL, F]
        r_c = res_b.results[c]["rs_o"][0]        # [NL]
        for j, blk in enumerate(_blocks_for_core(c)):
            rows = o_c[j * 32 : (j + 1) * 32] / r_c[j * 32 : (j + 1) * 32, None]
            out[blk * 32 : (blk + 1) * 32] = rows + pb_eff
    return out
